# revision 22
# baseline (speedup 1.0000x reference)
"""Trainium2 Bass kernel for nn_MergeDecoder (GNN message passing).

Distribution (8 NeuronCores):
  - stage 1 (memory-bound): child_W sharded expert-parallel over children ->
    8 children/core. child_W is split on the host into bf16 hi + bf16 lo
    (same total bytes as fp32, ~fp32 accuracy) and streamed through the PE
    as the moving matmul operand at 1 cycle/row against the stationary
    (p_hi, p_lo) parent pair; the two PSUM rows are folded on GpSimd.
  - x rows AllGathered in two halves (first half overlaps the second half
    of stage 1); node order is a fixed permutation undone on the host.
  - GIN MLPs tensor-parallel over the 2048 hidden dim (bf16 weights,
    FastWeightLoad), partial outputs AllReduced in fp32.
  - BatchNorm (batch stats over the 64 nodes) computed redundantly per core
    in transposed layout [feature, node] with batched 3D-AP vector ops.
"""

import numpy as np
import ml_dtypes

import concourse.bacc as bacc
import concourse.bass_isa as bass_isa
import concourse.mybir as mybir
import concourse.tile as tile
from concourse.bass_utils import run_bass_kernel_spmd

NCORES = 8
C = 64          # nodes (children)
F = 2048        # feature size
H = 2048        # hidden size
CPC = C // NCORES   # children per core
HALF = CPC // 2     # children per AllGather half
HS = H // NCORES    # hidden shard (tensor parallel)
KC = F // 128       # 128-row chunks of the feature dim
MS = HS // 128      # 128-row chunks of the hidden shard
BN_EPS = 1e-5

F32 = mybir.dt.float32
BF16 = mybir.dt.bfloat16
AX = mybir.AxisListType
ALU = mybir.AluOpType
ACT = mybir.ActivationFunctionType
BF = ml_dtypes.bfloat16

# gathered column i holds global child COLS_TO_CHILD[i] (AllGather halves)
COLS_TO_CHILD = [8 * r + j for h in (0, 1) for r in range(NCORES)
                 for j in range(HALF * h, HALF * h + HALF)]


def _3d(ap2d):
    """[128, KC*C] AP -> [128, KC, C]."""
    return ap2d.rearrange("p (k c) -> p k c", c=C)


def _bcast(ap2d_kc):
    """[128, KC] AP -> [128, KC, C] with a stride-0 broadcast last dim."""
    return ap2d_kc.unsqueeze(2).broadcast_to([128, KC, C])


def _gin_layer(nc, pools, hT_bf, wa_sb, ba_sb, wb_sb, wname):
    """T-layout tensor-parallel GIN MLP shard (bf16 weights/activations).

    returns gout [128, KC*C] f32: this rank's partial of
    relu(h @ Wa + ba) @ Wb, pre-AllReduce, no output bias.
    """
    acts, pmm = pools
    aT = acts.tile([128, MS * C], F32, name=f"aT_{wname}", tag="aT")
    for m in range(MS):
        pa = pmm.tile([128, C], F32, name="pa", tag="pa")
        for k in range(KC):
            nc.tensor.matmul(
                pa[:, :],
                lhsT=wa_sb[:, k * HS + m * 128 : k * HS + (m + 1) * 128],
                rhs=hT_bf[:, k * C : (k + 1) * C],
                start=(k == 0),
                stop=(k == KC - 1),
            )
        nc.scalar.activation(
            aT[:, m * C : (m + 1) * C], pa[:, :], ACT.Relu,
            bias=ba_sb[:, m : m + 1], scale=1.0,
        )
    gout = acts.tile([128, KC * C], F32, name=f"gout_{wname}", tag="gout")
    for m in range(KC):
        po = pmm.tile([128, C], F32, name="po", tag="po")
        for j in range(MS):
            nc.tensor.matmul(
                po[:, :],
                lhsT=wb_sb[:, j * F + m * 128 : j * F + (m + 1) * 128],
                rhs=aT[:, j * C : (j + 1) * C],
                start=(j == 0),
                stop=(j == MS - 1),
            )
        nc.scalar.copy(gout[:, m * C : (m + 1) * C], po[:, :])
    return gout


def _bias_relu_bn(nc, pools, ssum, bb_bcast, g_sb, be_sb, eps, name):
    """t = relu(ssum + b); y = BN(t), stats over the node (free) axis.

    Batched: mean/var via one Square + two 3D reductions; normalize via
    two broadcast-AP vector ops. Returns y [128, KC*C] f32.
    """
    acts, pmm = pools
    t = acts.tile([128, KC * C], F32, name=f"t_{name}", tag="t")
    sq = acts.tile([128, KC * C], F32, name=f"sq_{name}", tag="sq")
    nc.vector.scalar_tensor_tensor(
        sq[:, :], in0=ssum[:, :], scalar=1.0, in1=bb_bcast[:, :],
        op0=ALU.mult, op1=ALU.add,
    )
    nc.scalar.activation(t[:, :], sq[:, :], ACT.Relu)
    nc.scalar.square(sq[:, :], t[:, :])
    sums = acts.tile([128, KC], F32, name=f"sums_{name}")
    nc.vector.tensor_reduce(sums[:, :], _3d(t[:, :]), axis=AX.X, op=ALU.add)
    sumsq = acts.tile([128, KC], F32, name=f"sumsq_{name}")
    nc.vector.tensor_reduce(sumsq[:, :], _3d(sq[:, :]), axis=AX.X, op=ALU.add)
    mean = acts.tile([128, KC], F32, name=f"mean_{name}")
    nc.vector.tensor_scalar_mul(mean[:, :], in0=sums[:, :], scalar1=1.0 / C)
    var = acts.tile([128, KC], F32, name=f"var_{name}")
    nc.vector.scalar_tensor_tensor(   # (mean * -mean) + sumsq/C  = var
        var[:, :], in0=mean[:, :], scalar=-1.0, in1=mean[:, :],
        op0=ALU.mult, op1=ALU.mult,
    )
    nc.vector.scalar_tensor_tensor(
        var[:, :], in0=sumsq[:, :], scalar=1.0 / C, in1=var[:, :],
        op0=ALU.mult, op1=ALU.add,
    )
    std = acts.tile([128, KC], F32, name=f"std_{name}")
    nc.scalar.activation(std[:, :], var[:, :], ACT.Sqrt, bias=eps, scale=1.0)
    inv = acts.tile([128, KC], F32, name=f"inv_{name}")
    nc.vector.reciprocal(inv[:, :], std[:, :])
    scale = acts.tile([128, KC], F32, name=f"scale_{name}")
    nc.vector.scalar_tensor_tensor(
        scale[:, :], in0=inv[:, :], scalar=1.0, in1=g_sb[:, :],
        op0=ALU.mult, op1=ALU.mult,
    )
    shift = acts.tile([128, KC], F32, name=f"shift_{name}")
    nc.vector.scalar_tensor_tensor(
        shift[:, :], in0=mean[:, :], scalar=-1.0, in1=scale[:, :],
        op0=ALU.mult, op1=ALU.mult,
    )
    nc.vector.scalar_tensor_tensor(
        shift[:, :], in0=shift[:, :], scalar=1.0, in1=be_sb[:, :],
        op0=ALU.mult, op1=ALU.add,
    )
    y = acts.tile([128, KC * C], F32, name=f"y_{name}", tag="y")
    nc.vector.scalar_tensor_tensor(
        _3d(y[:, :]), in0=_3d(t[:, :]), scalar=1.0, in1=_bcast(scale[:, :]),
        op0=ALU.mult, op1=ALU.mult,
    )
    nc.vector.scalar_tensor_tensor(
        _3d(y[:, :]), in0=_3d(y[:, :]), scalar=1.0, in1=_bcast(shift[:, :]),
        op0=ALU.mult, op1=ALU.add,
    )
    return y


def _agg_h(nc, pools, xT_bf, agg, name):
    """h = x + agg broadcast over nodes, except node 0 (column 0) gets x only.
    agg: [128, KC] f32 AP. Returns hT bf16 [128, KC*C]."""
    acts, pmm = pools
    hT = acts.tile([128, KC * C], F32, name=f"hT_{name}", tag="hT")
    nc.vector.scalar_tensor_tensor(
        _3d(hT[:, :]), in0=_3d(xT_bf[:, :]), scalar=1.0, in1=_bcast(agg),
        op0=ALU.mult, op1=ALU.add,
    )
    nc.vector.tensor_scalar_add(
        _3d(hT[:, :])[:, :, 0:1], in0=_3d(xT_bf[:, :])[:, :, 0:1], scalar1=0.0)
    return hT


def build_nc():
    nc = bacc.Bacc("TRN2", target_bir_lowering=False, debug=False,
                   num_devices=NCORES)
    pT2_d = nc.dram_tensor("pT2", [128, 2 * KC], BF16, kind="ExternalInput")
    cwh_d = nc.dram_tensor("cwh", [CPC, F, F], BF16, kind="ExternalInput")
    cwl_d = nc.dram_tensor("cwl", [CPC, F, F], BF16, kind="ExternalInput")
    cb_d = nc.dram_tensor("cb", [CPC, F], F32, kind="ExternalInput")
    w1a_d = nc.dram_tensor("w1a", [F, HS], F32, kind="ExternalInput")
    b1a_d = nc.dram_tensor("b1a", [128, MS], F32, kind="ExternalInput")
    w1b_d = nc.dram_tensor("w1b", [HS, F], F32, kind="ExternalInput")
    b1bb_d = nc.dram_tensor("b1bb", [128, KC * C], F32, kind="ExternalInput")
    g1_d = nc.dram_tensor("g1", [128, KC], F32, kind="ExternalInput")
    be1_d = nc.dram_tensor("be1", [128, KC], F32, kind="ExternalInput")
    w2a_d = nc.dram_tensor("w2a", [F, HS], F32, kind="ExternalInput")
    b2a_d = nc.dram_tensor("b2a", [128, MS], F32, kind="ExternalInput")
    w2b_d = nc.dram_tensor("w2b", [HS, F], F32, kind="ExternalInput")
    b2bb_d = nc.dram_tensor("b2bb", [128, KC * C], F32, kind="ExternalInput")
    g2_d = nc.dram_tensor("g2", [128, KC], F32, kind="ExternalInput")
    be2_d = nc.dram_tensor("be2", [128, KC], F32, kind="ExternalInput")
    id_d = nc.dram_tensor("ident", [C, C], F32, kind="ExternalInput")
    out_d = nc.dram_tensor("outT", [128, KC, C], F32, kind="ExternalOutput")

    with tile.TileContext(nc) as tc:
        with (
            tc.tile_pool(name="consts", bufs=1) as consts,
            tc.tile_pool(name="ginw", bufs=1) as ginw,
            tc.tile_pool(name="acts", bufs=1) as acts,
            tc.tile_pool(name="dram", bufs=1, space="DRAM") as dram,
        ):
            # ---- constants + TP weight shards (prefetched during stage 1)
            pT2 = consts.tile([128, 2 * KC], BF16, name="pT2_sb")
            nc.sync.dma_start(pT2[:, :], pT2_d[:, :])
            ident = consts.tile([C, C], F32, name="ident_sb")
            nc.sync.dma_start(ident[:, :], id_d[:, :])
            eps_t = consts.tile([128, 1], F32, name="eps_sb")
            nc.gpsimd.memset(eps_t[:, :], BN_EPS)
            eps = eps_t[:, :]

            small = {}
            for nm, d in [("b1a", b1a_d), ("b1bb", b1bb_d), ("g1", g1_d),
                          ("be1", be1_d), ("b2a", b2a_d), ("b2bb", b2bb_d),
                          ("g2", g2_d), ("be2", be2_d)]:
                s = consts.tile(list(d.shape), F32, name=f"{nm}_sb")
                nc.sync.dma_start(s[:, :], d[:, :])
                small[nm] = s

            w1a = ginw.tile([128, KC * HS], F32, name="w1a_sb")
            w2a = ginw.tile([128, KC * HS], F32, name="w2a_sb")
            for k in range(KC):
                nc.sync.dma_start(w1a[:, k * HS : (k + 1) * HS],
                                  w1a_d[k * 128 : (k + 1) * 128, :])
                nc.sync.dma_start(w2a[:, k * HS : (k + 1) * HS],
                                  w2a_d[k * 128 : (k + 1) * 128, :])
            w1b = ginw.tile([128, MS * F], F32, name="w1b_sb")
            w2b = ginw.tile([128, MS * F], F32, name="w2b_sb")
            for j in range(MS):
                nc.sync.dma_start(w1b[:, j * F : (j + 1) * F],
                                  w1b_d[j * 128 : (j + 1) * 128, :])
                nc.sync.dma_start(w2b[:, j * F : (j + 1) * F],
                                  w2b_d[j * 128 : (j + 1) * 128, :])

            # ---- stage 1: x[c] = relu(parent @ child_W[c] + child_b[c])
            # bf16 hi/lo split: psum row0 = p_hi @ (Whi+Wlo),
            # row1 = p_lo @ (Whi+Wlo); fold rows on gpsimd.
            ag_in = {}
            ag_out = {}
            for hname, hh in (("A", 0), ("B", 1)):
                ag_in[hh] = dram.tile([HALF, F], F32, name=f"ag{hname}_in")
                ag_out[hh] = dram.tile([HALF * NCORES, F], F32,
                                       name=f"ag{hname}_out",
                                       addr_space="Shared")
            with (
                tc.tile_pool(name="whi", bufs=5) as whip,
                tc.tile_pool(name="wlo", bufs=5) as wlop,
                tc.tile_pool(name="prow", bufs=2, space="PSUM") as prow,
                tc.tile_pool(name="s1rows", bufs=2) as s1rows,
                tc.tile_pool(name="s1rows2", bufs=1) as s1rows2,
            ):
                for j in range(CPC):
                    half = j // HALF
                    pr = prow.tile([2, F], F32, name="pr", tag="pr")
                    for k in range(KC):
                        wh = whip.tile([128, F], BF16, name="wh", tag="wh")
                        nc.sync.dma_start(
                            wh[:, :], cwh_d[j, k * 128 : (k + 1) * 128, :])
                        wl = wlop.tile([128, F], BF16, name="wl", tag="wl")
                        nc.sync.dma_start(
                            wl[:, :], cwl_d[j, k * 128 : (k + 1) * 128, :])
                        for si, wt in ((0, wh), (1, wl)):
                            for n in range(F // 512):
                                nc.tensor.matmul(
                                    pr[0:2, n * 512 : (n + 1) * 512],
                                    lhsT=pT2[:, 2 * k : 2 * k + 2],
                                    rhs=wt[:, n * 512 : (n + 1) * 512],
                                    start=(k == 0 and si == 0),
                                    stop=(k == KC - 1 and si == 1),
                                )
                    t2 = s1rows.tile([2, F], F32, name="t2", tag="t2")
                    nc.vector.tensor_scalar_add(
                        t2[:, :], in0=pr[0:2, :], scalar1=0.0)
                    xs = s1rows.tile([2, F], F32, name="xs", tag="xs")
                    nc.gpsimd.partition_all_reduce(
                        xs[0:2, :], t2[0:2, :], channels=2,
                        reduce_op=bass_isa.ReduceOp.add)
                    cbrow = s1rows2.tile([1, F], F32, name="cbrow", tag="cbr")
                    nc.sync.dma_start(cbrow[:, :], cb_d[j : j + 1, :])
                    nc.vector.scalar_tensor_tensor(
                        xs[0:1, :], in0=xs[0:1, :], scalar=1.0,
                        in1=cbrow[:, :], op0=ALU.mult, op1=ALU.add,
                    )
                    xrow = s1rows2.tile([1, F], F32, name="xrow", tag="xr")
                    nc.scalar.activation(xrow[:, :], xs[0:1, :], ACT.Relu)
                    nc.sync.dma_start(
                        ag_in[half][j % HALF : j % HALF + 1, :], xrow[:, :])
                    if j % HALF == HALF - 1:
                        nc.gpsimd.collective_compute(
                            "AllGather", ALU.bypass,
                            replica_groups=[list(range(NCORES))],
                            ins=[ag_in[half][:, :].opt()],
                            outs=[ag_out[half][:, :].opt()],
                        )

            xfull = acts.tile([C, F], F32, name="xfull_sb")
            for hh in (0, 1):
                nc.sync.dma_start(
                    xfull[hh * HALF * NCORES : (hh + 1) * HALF * NCORES, :],
                    ag_out[hh][:, :])

            with tc.tile_pool(name="pmm", bufs=2, space="PSUM") as pmm:
                pools = (acts, pmm)
                # transpose to T-layout xT [feature, node] (bf16)
                xT = acts.tile([128, KC * C], F32, name="xT_sb")
                for k in range(KC):
                    ptx = pmm.tile([128, C], F32, name="ptx", tag="ptx")
                    nc.tensor.transpose(
                        ptx[:, :], xfull[:, k * 128 : (k + 1) * 128],
                        ident[:, :])
                    nc.scalar.copy(xT[:, k * C : (k + 1) * C], ptx[:, :])

                # ---- GIN layer 1
                agg1 = acts.tile([128, KC], F32, name="agg1")
                nc.vector.tensor_reduce(
                    agg1[:, :], _3d(xT[:, :]), axis=AX.X, op=ALU.add)
                h1 = _agg_h(nc, pools, xT, agg1[:, :], "l1")
                g1out = _gin_layer(nc, pools, h1, w1a, small["b1a"],
                                   w1b, "l1")
                ar1_in = dram.tile([128, KC * C], F32, name="ar1_in")
                ar1_out = dram.tile([128, KC * C], F32, name="ar1_out",
                                    addr_space="Shared")
                nc.sync.dma_start(ar1_in[:, :], g1out[:, :])
                nc.gpsimd.collective_compute(
                    "AllReduce", ALU.add,
                    replica_groups=[list(range(NCORES))],
                    ins=[ar1_in[:, :].opt()], outs=[ar1_out[:, :].opt()],
                )
                s1 = acts.tile([128, KC * C], F32, name="s1_sb", tag="s")
                nc.sync.dma_start(s1[:, :], ar1_out[:, :])
                y1 = _bias_relu_bn(nc, pools, s1, small["b1bb"],
                                   small["g1"], small["be1"], eps, "l1")

                # ---- GIN layer 2; agg over BN output is 64*beta1 exactly
                agg2 = acts.tile([128, KC], F32, name="agg2")
                nc.vector.tensor_scalar_mul(
                    agg2[:, :], in0=small["be1"][:, :], scalar1=float(C))
                h2 = _agg_h(nc, pools, y1, agg2[:, :], "l2")
                g2out = _gin_layer(nc, pools, h2, w2a, small["b2a"],
                                   w2b, "l2")
                ar2_in = dram.tile([128, KC * C], F32, name="ar2_in")
                ar2_out = dram.tile([128, KC * C], F32, name="ar2_out",
                                    addr_space="Shared")
                nc.sync.dma_start(ar2_in[:, :], g2out[:, :])
                nc.gpsimd.collective_compute(
                    "AllReduce", ALU.add,
                    replica_groups=[list(range(NCORES))],
                    ins=[ar2_in[:, :].opt()], outs=[ar2_out[:, :].opt()],
                )
                s2 = acts.tile([128, KC * C], F32, name="s2_sb", tag="s")
                nc.sync.dma_start(s2[:, :], ar2_out[:, :])
                y2 = _bias_relu_bn(nc, pools, s2, small["b2bb"],
                                   small["g2"], small["be2"], eps, "l2")

                nc.sync.dma_start(out_d[:, :, :], y2[:, :])
    nc.finalize()
    return nc


def _colmajor(v, cols):
    """[cols*128] vector -> [128, cols] with column k = v[k*128:(k+1)*128]."""
    return np.ascontiguousarray(np.asarray(v, np.float32).reshape(cols, 128).T)


def _split_bf16(a):
    hi = a.astype(BF)
    lo = (a - hi.astype(np.float32)).astype(BF)
    return hi, lo


def prepare_in_maps(inputs):
    f32 = np.float32
    parent = np.asarray(inputs["parent_feature"], f32).reshape(-1)
    child_W = np.asarray(inputs["child_W"], f32)
    child_b = np.asarray(inputs["child_b"], f32)

    p_hi, p_lo = _split_bf16(parent)
    pT2 = np.empty((128, 2 * KC), BF)
    pT2[:, 0::2] = p_hi.reshape(KC, 128).T
    pT2[:, 1::2] = p_lo.reshape(KC, 128).T

    ident = np.eye(C, dtype=f32)
    b1bb = np.repeat(_colmajor(inputs["b1b"], KC)[:, :, None], C,
                     axis=2).reshape(128, KC * C)
    b2bb = np.repeat(_colmajor(inputs["b2b"], KC)[:, :, None], C,
                     axis=2).reshape(128, KC * C)
    g1T = _colmajor(inputs["g1"], KC)
    be1T = _colmajor(inputs["beta1"], KC)
    g2T = _colmajor(inputs["g2"], KC)
    be2T = _colmajor(inputs["beta2"], KC)
    b1a = np.asarray(inputs["b1a"], f32)
    b2a = np.asarray(inputs["b2a"], f32)
    W1a = np.asarray(inputs["W1a"], f32)
    W1b = np.asarray(inputs["W1b"], f32)
    W2a = np.asarray(inputs["W2a"], f32)
    W2b = np.asarray(inputs["W2b"], f32)

    in_maps = []
    for r in range(NCORES):
        sl = slice(r * HS, (r + 1) * HS)
        cw = np.ascontiguousarray(child_W[r * CPC : (r + 1) * CPC])
        cwh, cwl = _split_bf16(cw)
        in_maps.append({
            "pT2": pT2,
            "cwh": cwh,
            "cwl": cwl,
            "cb": np.ascontiguousarray(child_b[r * CPC : (r + 1) * CPC]),
            "w1a": np.ascontiguousarray(W1a[:, sl]),
            "b1a": _colmajor(b1a[sl], MS),
            "w1b": np.ascontiguousarray(W1b[sl, :]),
            "b1bb": b1bb, "g1": g1T, "be1": be1T,
            "w2a": np.ascontiguousarray(W2a[:, sl]),
            "b2a": _colmajor(b2a[sl], MS),
            "w2b": np.ascontiguousarray(W2b[sl, :]),
            "b2bb": b2bb, "g2": g2T, "be2": be2T,
            "ident": ident,
        })
    return in_maps


_NC_CACHE = {}


def get_nc():
    if "nc" not in _NC_CACHE:
        _NC_CACHE["nc"] = build_nc()
    return _NC_CACHE["nc"]


def unpack_out(outT):
    # outT [128, KC, C]: outT[p, k, i] = out[COLS_TO_CHILD[i], k*128 + p]
    cols = np.asarray(outT).transpose(2, 1, 0).reshape(C, F)  # [i, f]
    out = np.empty((C, F), np.float32)
    out[np.array(COLS_TO_CHILD)] = cols
    return out


def kernel(**inputs):
    nc = get_nc()
    in_maps = prepare_in_maps(inputs)
    res = run_bass_kernel_spmd(nc, in_maps, core_ids=list(range(NCORES)))
    return unpack_out(res.results[0]["outT"])


# revision 24
# speedup vs baseline: 1.0887x; 1.0887x over previous
"""Trainium2 Bass kernel for nn_MergeDecoder (GNN message passing).

Distribution (8 NeuronCores):
  - stage 1 (memory-bound): child_W sharded expert-parallel over children ->
    8 children/core. child_W is split on the host into bf16 hi + bf16 lo
    (same total bytes as fp32, ~fp32 accuracy) and streamed through the PE
    as the moving matmul operand at 1 cycle/row against the stationary
    (p_hi, p_lo) parent pair; the two PSUM rows are folded on GpSimd.
  - x rows AllGathered in two halves (first half overlaps the second half
    of stage 1); node order is a fixed permutation undone on the host.
  - GIN MLPs tensor-parallel over the 2048 hidden dim (bf16 weights,
    FastWeightLoad), partial outputs AllReduced in fp32.
  - BatchNorm (batch stats over the 64 nodes) computed redundantly per core
    in transposed layout [feature, node] with batched 3D-AP vector ops.
"""

import numpy as np
import ml_dtypes

import concourse.bacc as bacc
import concourse.bass_isa as bass_isa
import concourse.mybir as mybir
import concourse.tile as tile
from concourse.bass_utils import run_bass_kernel_spmd

NCORES = 8
C = 64          # nodes (children)
F = 2048        # feature size
H = 2048        # hidden size
CPC = C // NCORES   # children per core
HALF = CPC // 2     # children per AllGather half
HS = H // NCORES    # hidden shard (tensor parallel)
KC = F // 128       # 128-row chunks of the feature dim
MS = HS // 128      # 128-row chunks of the hidden shard
BN_EPS = 1e-5

F32 = mybir.dt.float32
BF16 = mybir.dt.bfloat16
AX = mybir.AxisListType
ALU = mybir.AluOpType
ACT = mybir.ActivationFunctionType
BF = ml_dtypes.bfloat16

# gathered column i holds global child COLS_TO_CHILD[i] (AllGather halves)
COLS_TO_CHILD = [8 * r + j for h in (0, 1) for r in range(NCORES)
                 for j in range(HALF * h, HALF * h + HALF)]


def _3d(ap2d):
    """[128, KC*C] AP -> [128, KC, C]."""
    return ap2d.rearrange("p (k c) -> p k c", c=C)


def _bcast(ap2d_kc):
    """[128, KC] AP -> [128, KC, C] with a stride-0 broadcast last dim."""
    return ap2d_kc.unsqueeze(2).broadcast_to([128, KC, C])


def _gin_layer(nc, pools, hT_bf, wa_sb, ba_sb, wb_sb, wname):
    """T-layout tensor-parallel GIN MLP shard (bf16 weights/activations).

    returns gout [128, KC*C] f32: this rank's partial of
    relu(h @ Wa + ba) @ Wb, pre-AllReduce, no output bias.
    """
    acts, pmm = pools
    aT = acts.tile([128, MS * C], F32, name=f"aT_{wname}")
    for m in range(MS):
        pa = pmm.tile([128, C], F32, name="pa", tag="pa")
        for k in range(KC):
            nc.tensor.matmul(
                pa[:, :],
                lhsT=wa_sb[:, k * HS + m * 128 : k * HS + (m + 1) * 128],
                rhs=hT_bf[:, k * C : (k + 1) * C],
                start=(k == 0),
                stop=(k == KC - 1),
            )
        nc.scalar.activation(
            aT[:, m * C : (m + 1) * C], pa[:, :], ACT.Relu,
            bias=ba_sb[:, m : m + 1], scale=1.0,
        )
    gout = acts.tile([128, KC * C], F32, name=f"gout_{wname}")
    for m in range(KC):
        po = pmm.tile([128, C], F32, name="po", tag="po")
        for j in range(MS):
            nc.tensor.matmul(
                po[:, :],
                lhsT=wb_sb[:, j * F + m * 128 : j * F + (m + 1) * 128],
                rhs=aT[:, j * C : (j + 1) * C],
                start=(j == 0),
                stop=(j == MS - 1),
            )
        nc.scalar.copy(gout[:, m * C : (m + 1) * C], po[:, :])
    return gout


def _bias_relu_bn(nc, pools, ssum, bb_bcast, g_sb, be_sb, eps, name):
    """t = relu(ssum + b); y = BN(t), stats over the node (free) axis.

    Batched: mean/var via one Square + two 3D reductions; normalize via
    two broadcast-AP vector ops. Returns y [128, KC*C] f32.
    """
    acts, pmm = pools
    t = acts.tile([128, KC * C], F32, name=f"t_{name}")
    sq = acts.tile([128, KC * C], F32, name=f"sq_{name}", tag="sq")
    nc.vector.scalar_tensor_tensor(
        sq[:, :], in0=ssum[:, :], scalar=1.0, in1=bb_bcast[:, :],
        op0=ALU.mult, op1=ALU.add,
    )
    nc.scalar.activation(t[:, :], sq[:, :], ACT.Relu)
    nc.scalar.square(sq[:, :], t[:, :])
    sums = acts.tile([128, KC], F32, name=f"sums_{name}")
    nc.vector.tensor_reduce(sums[:, :], _3d(t[:, :]), axis=AX.X, op=ALU.add)
    sumsq = acts.tile([128, KC], F32, name=f"sumsq_{name}")
    nc.vector.tensor_reduce(sumsq[:, :], _3d(sq[:, :]), axis=AX.X, op=ALU.add)
    mean = acts.tile([128, KC], F32, name=f"mean_{name}")
    nc.vector.tensor_scalar_mul(mean[:, :], in0=sums[:, :], scalar1=1.0 / C)
    var = acts.tile([128, KC], F32, name=f"var_{name}")
    nc.vector.scalar_tensor_tensor(   # (mean * -mean) + sumsq/C  = var
        var[:, :], in0=mean[:, :], scalar=-1.0, in1=mean[:, :],
        op0=ALU.mult, op1=ALU.mult,
    )
    nc.vector.scalar_tensor_tensor(
        var[:, :], in0=sumsq[:, :], scalar=1.0 / C, in1=var[:, :],
        op0=ALU.mult, op1=ALU.add,
    )
    std = acts.tile([128, KC], F32, name=f"std_{name}")
    nc.scalar.activation(std[:, :], var[:, :], ACT.Sqrt, bias=eps, scale=1.0)
    inv = acts.tile([128, KC], F32, name=f"inv_{name}")
    nc.vector.reciprocal(inv[:, :], std[:, :])
    scale = acts.tile([128, KC], F32, name=f"scale_{name}")
    nc.vector.scalar_tensor_tensor(
        scale[:, :], in0=inv[:, :], scalar=1.0, in1=g_sb[:, :],
        op0=ALU.mult, op1=ALU.mult,
    )
    shift = acts.tile([128, KC], F32, name=f"shift_{name}")
    nc.vector.scalar_tensor_tensor(
        shift[:, :], in0=mean[:, :], scalar=-1.0, in1=scale[:, :],
        op0=ALU.mult, op1=ALU.mult,
    )
    nc.vector.scalar_tensor_tensor(
        shift[:, :], in0=shift[:, :], scalar=1.0, in1=be_sb[:, :],
        op0=ALU.mult, op1=ALU.add,
    )
    y = acts.tile([128, KC * C], F32, name=f"y_{name}")
    nc.vector.scalar_tensor_tensor(
        _3d(y[:, :]), in0=_3d(t[:, :]), scalar=1.0, in1=_bcast(scale[:, :]),
        op0=ALU.mult, op1=ALU.mult,
    )
    nc.vector.scalar_tensor_tensor(
        _3d(y[:, :]), in0=_3d(y[:, :]), scalar=1.0, in1=_bcast(shift[:, :]),
        op0=ALU.mult, op1=ALU.add,
    )
    return y


def _agg_h(nc, pools, xT_bf, agg, name):
    """h = x + agg broadcast over nodes, except node 0 (column 0) gets x only.
    agg: [128, KC] f32 AP. Returns hT bf16 [128, KC*C]."""
    acts, pmm = pools
    hT = acts.tile([128, KC * C], F32, name=f"hT_{name}")
    nc.vector.scalar_tensor_tensor(
        _3d(hT[:, :]), in0=_3d(xT_bf[:, :]), scalar=1.0, in1=_bcast(agg),
        op0=ALU.mult, op1=ALU.add,
    )
    nc.vector.tensor_scalar_add(
        _3d(hT[:, :])[:, :, 0:1], in0=_3d(xT_bf[:, :])[:, :, 0:1], scalar1=0.0)
    return hT


def build_nc():
    nc = bacc.Bacc("TRN2", target_bir_lowering=False, debug=False,
                   num_devices=NCORES)
    pT2_d = nc.dram_tensor("pT2", [128, 2 * KC], BF16, kind="ExternalInput")
    cwh_d = nc.dram_tensor("cwh", [CPC, F, F], BF16, kind="ExternalInput")
    cwl_d = nc.dram_tensor("cwl", [CPC, F, F], BF16, kind="ExternalInput")
    cb_d = nc.dram_tensor("cb", [CPC, F], F32, kind="ExternalInput")
    w1a_d = nc.dram_tensor("w1a", [F, HS], F32, kind="ExternalInput")
    b1a_d = nc.dram_tensor("b1a", [128, MS], F32, kind="ExternalInput")
    w1b_d = nc.dram_tensor("w1b", [HS, F], F32, kind="ExternalInput")
    b1bb_d = nc.dram_tensor("b1bb", [128, KC * C], F32, kind="ExternalInput")
    g1_d = nc.dram_tensor("g1", [128, KC], F32, kind="ExternalInput")
    be1_d = nc.dram_tensor("be1", [128, KC], F32, kind="ExternalInput")
    w2a_d = nc.dram_tensor("w2a", [F, HS], F32, kind="ExternalInput")
    b2a_d = nc.dram_tensor("b2a", [128, MS], F32, kind="ExternalInput")
    w2b_d = nc.dram_tensor("w2b", [HS, F], F32, kind="ExternalInput")
    b2bb_d = nc.dram_tensor("b2bb", [128, KC * C], F32, kind="ExternalInput")
    g2_d = nc.dram_tensor("g2", [128, KC], F32, kind="ExternalInput")
    be2_d = nc.dram_tensor("be2", [128, KC], F32, kind="ExternalInput")
    id_d = nc.dram_tensor("ident", [C, C], F32, kind="ExternalInput")
    out_d = nc.dram_tensor("outT", [128, KC, C], F32, kind="ExternalOutput")

    with tile.TileContext(nc) as tc:
        with (
            tc.tile_pool(name="consts", bufs=1) as consts,
            tc.tile_pool(name="ginw", bufs=1) as ginw,
            tc.tile_pool(name="acts", bufs=1) as acts,
            tc.tile_pool(name="dram", bufs=1, space="DRAM") as dram,
        ):
            # ---- constants + TP weight shards (prefetched during stage 1)
            pT2 = consts.tile([128, 2 * KC], BF16, name="pT2_sb")
            nc.sync.dma_start(pT2[:, :], pT2_d[:, :])
            ident = consts.tile([C, C], F32, name="ident_sb")
            nc.sync.dma_start(ident[:, :], id_d[:, :])
            eps_t = consts.tile([128, 1], F32, name="eps_sb")
            nc.gpsimd.memset(eps_t[:, :], BN_EPS)
            eps = eps_t[:, :]
            ones2 = consts.tile([2, 1], F32, name="ones2_sb")
            nc.gpsimd.memset(ones2[:, :], 1.0)

            small = {}
            for nm, d in [("b1a", b1a_d), ("b1bb", b1bb_d), ("g1", g1_d),
                          ("be1", be1_d), ("b2a", b2a_d), ("b2bb", b2bb_d),
                          ("g2", g2_d), ("be2", be2_d)]:
                s = consts.tile(list(d.shape), F32, name=f"{nm}_sb")
                nc.sync.dma_start(s[:, :], d[:, :])
                small[nm] = s

            w1a = ginw.tile([128, KC * HS], F32, name="w1a_sb")
            w2a = ginw.tile([128, KC * HS], F32, name="w2a_sb")
            for k in range(KC):
                nc.sync.dma_start(w1a[:, k * HS : (k + 1) * HS],
                                  w1a_d[k * 128 : (k + 1) * 128, :])
                nc.sync.dma_start(w2a[:, k * HS : (k + 1) * HS],
                                  w2a_d[k * 128 : (k + 1) * 128, :])
            w1b = ginw.tile([128, MS * F], F32, name="w1b_sb")
            w2b = ginw.tile([128, MS * F], F32, name="w2b_sb")
            for j in range(MS):
                nc.sync.dma_start(w1b[:, j * F : (j + 1) * F],
                                  w1b_d[j * 128 : (j + 1) * 128, :])
                nc.sync.dma_start(w2b[:, j * F : (j + 1) * F],
                                  w2b_d[j * 128 : (j + 1) * 128, :])

            # ---- stage 1: x[c] = relu(parent @ child_W[c] + child_b[c])
            # bf16 hi/lo split: psum row0 = p_hi @ (Whi+Wlo),
            # row1 = p_lo @ (Whi+Wlo); fold rows on gpsimd.
            ag_in = {}
            ag_out = {}
            for hname, hh in (("A", 0), ("B", 1)):
                ag_in[hh] = dram.tile([HALF, F], F32, name=f"ag{hname}_in")
                ag_out[hh] = dram.tile([HALF * NCORES, F], F32,
                                       name=f"ag{hname}_out",
                                       addr_space="Shared")
            with (
                tc.tile_pool(name="whi", bufs=5) as whip,
                tc.tile_pool(name="wlo", bufs=5) as wlop,
                tc.tile_pool(name="prow", bufs=2, space="PSUM") as prow,
                tc.tile_pool(name="s1rows", bufs=1) as s1rows,
                tc.tile_pool(name="s1rows2", bufs=1) as s1rows2,
            ):
                for j in range(CPC):
                    half = j // HALF
                    pr = prow.tile([2, F], F32, name="pr", tag="pr")
                    for k in range(KC):
                        wh = whip.tile([128, F], BF16, name="wh", tag="wh")
                        nc.sync.dma_start(
                            wh[:, :], cwh_d[j, k * 128 : (k + 1) * 128, :])
                        wl = wlop.tile([128, F], BF16, name="wl", tag="wl")
                        nc.sync.dma_start(
                            wl[:, :], cwl_d[j, k * 128 : (k + 1) * 128, :])
                        for si, wt in ((0, wh), (1, wl)):
                            for n in range(F // 512):
                                nc.tensor.matmul(
                                    pr[0:2, n * 512 : (n + 1) * 512],
                                    lhsT=pT2[:, 2 * k : 2 * k + 2],
                                    rhs=wt[:, n * 512 : (n + 1) * 512],
                                    start=(k == 0 and si == 0),
                                    stop=(k == KC - 1 and si == 1),
                                )
                    t2 = s1rows.tile([2, F], F32, name="t2", tag="t2")
                    nc.vector.tensor_scalar_add(
                        t2[:, :], in0=pr[0:2, :], scalar1=0.0)
                    for n in range(F // 512):
                        nc.tensor.matmul(
                            pr[0:1, n * 512 : (n + 1) * 512],
                            lhsT=ones2[0:2, 0:1],
                            rhs=t2[0:2, n * 512 : (n + 1) * 512],
                            start=True, stop=True,
                        )
                    cbrow = s1rows2.tile([1, F], F32, name="cbrow", tag="cbr")
                    nc.sync.dma_start(cbrow[:, :], cb_d[j : j + 1, :])
                    xs = s1rows.tile([1, F], F32, name="xs", tag="xs")
                    nc.vector.scalar_tensor_tensor(
                        xs[0:1, :], in0=pr[0:1, :], scalar=1.0,
                        in1=cbrow[:, :], op0=ALU.mult, op1=ALU.add,
                    )
                    xrow = s1rows2.tile([1, F], F32, name="xrow", tag="xr")
                    nc.scalar.activation(xrow[:, :], xs[0:1, :], ACT.Relu)
                    nc.sync.dma_start(
                        ag_in[half][j % HALF : j % HALF + 1, :], xrow[:, :])
                    if j % HALF == HALF - 1:
                        nc.gpsimd.collective_compute(
                            "AllGather", ALU.bypass,
                            replica_groups=[list(range(NCORES))],
                            ins=[ag_in[half][:, :].opt()],
                            outs=[ag_out[half][:, :].opt()],
                        )

            xfull = acts.tile([C, F], F32, name="xfull_sb")
            for hh in (0, 1):
                nc.sync.dma_start(
                    xfull[hh * HALF * NCORES : (hh + 1) * HALF * NCORES, :],
                    ag_out[hh][:, :])

            with tc.tile_pool(name="pmm", bufs=2, space="PSUM") as pmm:
                pools = (acts, pmm)
                # transpose to T-layout xT [feature, node] (bf16)
                xT = acts.tile([128, KC * C], F32, name="xT_sb")
                for k in range(KC):
                    ptx = pmm.tile([128, C], F32, name="ptx", tag="ptx")
                    nc.tensor.transpose(
                        ptx[:, :], xfull[:, k * 128 : (k + 1) * 128],
                        ident[:, :])
                    nc.scalar.copy(xT[:, k * C : (k + 1) * C], ptx[:, :])

                # ---- GIN layer 1
                agg1 = acts.tile([128, KC], F32, name="agg1")
                nc.vector.tensor_reduce(
                    agg1[:, :], _3d(xT[:, :]), axis=AX.X, op=ALU.add)
                h1 = _agg_h(nc, pools, xT, agg1[:, :], "l1")
                g1out = _gin_layer(nc, pools, h1, w1a, small["b1a"],
                                   w1b, "l1")
                ar1_in = dram.tile([128, KC * C], F32, name="ar1_in")
                ar1_out = dram.tile([128, KC * C], F32, name="ar1_out",
                                    addr_space="Shared")
                nc.sync.dma_start(ar1_in[:, :], g1out[:, :])
                nc.gpsimd.collective_compute(
                    "AllReduce", ALU.add,
                    replica_groups=[list(range(NCORES))],
                    ins=[ar1_in[:, :].opt()], outs=[ar1_out[:, :].opt()],
                )
                s1 = acts.tile([128, KC * C], F32, name="s1_sb")
                nc.sync.dma_start(s1[:, :], ar1_out[:, :])
                y1 = _bias_relu_bn(nc, pools, s1, small["b1bb"],
                                   small["g1"], small["be1"], eps, "l1")

                # ---- GIN layer 2; agg over BN output is 64*beta1 exactly
                agg2 = acts.tile([128, KC], F32, name="agg2")
                nc.vector.tensor_scalar_mul(
                    agg2[:, :], in0=small["be1"][:, :], scalar1=float(C))
                h2 = _agg_h(nc, pools, y1, agg2[:, :], "l2")
                g2out = _gin_layer(nc, pools, h2, w2a, small["b2a"],
                                   w2b, "l2")
                ar2_in = dram.tile([128, KC * C], F32, name="ar2_in")
                ar2_out = dram.tile([128, KC * C], F32, name="ar2_out",
                                    addr_space="Shared")
                nc.sync.dma_start(ar2_in[:, :], g2out[:, :])
                nc.gpsimd.collective_compute(
                    "AllReduce", ALU.add,
                    replica_groups=[list(range(NCORES))],
                    ins=[ar2_in[:, :].opt()], outs=[ar2_out[:, :].opt()],
                )
                s2 = acts.tile([128, KC * C], F32, name="s2_sb")
                nc.sync.dma_start(s2[:, :], ar2_out[:, :])
                y2 = _bias_relu_bn(nc, pools, s2, small["b2bb"],
                                   small["g2"], small["be2"], eps, "l2")

                nc.sync.dma_start(out_d[:, :, :], y2[:, :])
    nc.finalize()
    return nc


def _colmajor(v, cols):
    """[cols*128] vector -> [128, cols] with column k = v[k*128:(k+1)*128]."""
    return np.ascontiguousarray(np.asarray(v, np.float32).reshape(cols, 128).T)


def _split_bf16(a):
    hi = a.astype(BF)
    lo = (a - hi.astype(np.float32)).astype(BF)
    return hi, lo


def prepare_in_maps(inputs):
    f32 = np.float32
    parent = np.asarray(inputs["parent_feature"], f32).reshape(-1)
    child_W = np.asarray(inputs["child_W"], f32)
    child_b = np.asarray(inputs["child_b"], f32)

    p_hi, p_lo = _split_bf16(parent)
    pT2 = np.empty((128, 2 * KC), BF)
    pT2[:, 0::2] = p_hi.reshape(KC, 128).T
    pT2[:, 1::2] = p_lo.reshape(KC, 128).T

    ident = np.eye(C, dtype=f32)
    b1bb = np.repeat(_colmajor(inputs["b1b"], KC)[:, :, None], C,
                     axis=2).reshape(128, KC * C)
    b2bb = np.repeat(_colmajor(inputs["b2b"], KC)[:, :, None], C,
                     axis=2).reshape(128, KC * C)
    g1T = _colmajor(inputs["g1"], KC)
    be1T = _colmajor(inputs["beta1"], KC)
    g2T = _colmajor(inputs["g2"], KC)
    be2T = _colmajor(inputs["beta2"], KC)
    b1a = np.asarray(inputs["b1a"], f32)
    b2a = np.asarray(inputs["b2a"], f32)
    W1a = np.asarray(inputs["W1a"], f32)
    W1b = np.asarray(inputs["W1b"], f32)
    W2a = np.asarray(inputs["W2a"], f32)
    W2b = np.asarray(inputs["W2b"], f32)

    in_maps = []
    for r in range(NCORES):
        sl = slice(r * HS, (r + 1) * HS)
        cw = np.ascontiguousarray(child_W[r * CPC : (r + 1) * CPC])
        cwh, cwl = _split_bf16(cw)
        in_maps.append({
            "pT2": pT2,
            "cwh": cwh,
            "cwl": cwl,
            "cb": np.ascontiguousarray(child_b[r * CPC : (r + 1) * CPC]),
            "w1a": np.ascontiguousarray(W1a[:, sl]),
            "b1a": _colmajor(b1a[sl], MS),
            "w1b": np.ascontiguousarray(W1b[sl, :]),
            "b1bb": b1bb, "g1": g1T, "be1": be1T,
            "w2a": np.ascontiguousarray(W2a[:, sl]),
            "b2a": _colmajor(b2a[sl], MS),
            "w2b": np.ascontiguousarray(W2b[sl, :]),
            "b2bb": b2bb, "g2": g2T, "be2": be2T,
            "ident": ident,
        })
    return in_maps


_NC_CACHE = {}


def get_nc():
    if "nc" not in _NC_CACHE:
        _NC_CACHE["nc"] = build_nc()
    return _NC_CACHE["nc"]


def unpack_out(outT):
    # outT [128, KC, C]: outT[p, k, i] = out[COLS_TO_CHILD[i], k*128 + p]
    cols = np.asarray(outT).transpose(2, 1, 0).reshape(C, F)  # [i, f]
    out = np.empty((C, F), np.float32)
    out[np.array(COLS_TO_CHILD)] = cols
    return out


def kernel(**inputs):
    nc = get_nc()
    in_maps = prepare_in_maps(inputs)
    res = run_bass_kernel_spmd(nc, in_maps, core_ids=list(range(NCORES)))
    return unpack_out(res.results[0]["outT"])
